# revision 2
# baseline (speedup 1.0000x reference)
"""Trainium2 Bass kernel for nn_DecoderCell (LFADS-style decoder cell).

Data-parallel over 8 NeuronCores: batch 32768 -> 4096 rows/core, weights
replicated. Activations feature-major ([feat, batch]); host pre-transposes
inputs / post-transposes outputs.

V2: generator-GRU hidden matmuls + factor matmul run in fp8(e4m3) with
perf_mode=DoubleRow (K=256 per pass, ~1.8x tensor throughput); controller
GRU + co-projection stay bf16 (std=exp(logvar/2) amplifies errors, so the
con path needs the precision). gen-ih (K=64) matmuls are packed pairwise
into row-groups (0,0)/(64,0) so two run concurrently. co mean/logvar are
packed into one 128-wide PSUM group. All elementwise is bf16 (2x DVE),
all DMA I/O is bf16 (or fp8 for the DoubleRow operand copies).

fp8 weights are pre-scaled x16 on host (the matching ih weights too, so
PSUM groups stay on one scale); the 1/16 folds into activation scales.
Gate sigmoids are tanh-form (sigmoid(x) = 0.5 + 0.5*tanh(x/2)) as in V1.
"""

import numpy as np

B, CI, GEN, CON, CO, FAC = 32768, 128, 512, 256, 64, 128
NCORES = 8
BS = B // NCORES            # 4096 rows per core
NT = 512                    # batch columns per tile
NTILES = BS // NT           # 8
CLIP = 5.0
XIN = 2 * CI + FAC          # 384 controller input features

_CACHE = {}


def _build(reps=1, nt=None, knobs=None):
    from contextlib import ExitStack
    import concourse.bacc as bacc
    import concourse.tile as tile
    from concourse import mybir
    from concourse.bass_interp import get_hw_module

    F32 = mybir.dt.float32
    BF16 = mybir.dt.bfloat16
    FP8 = mybir.dt.float8e4
    AF = mybir.ActivationFunctionType
    OP = mybir.AluOpType
    DR = mybir.MatmulPerfMode.DoubleRow

    NT = nt or globals()["NT"]
    NTILES = BS // NT
    kn = {"pcon": 3, "pgen": 4, "psco": 1, "clip_eng": "vector",
          "io_bufs": 2, "zm_eng": "gpsimd"}
    kn.update(knobs or {})
    nc = bacc.Bacc("TRN2", debug=False, target_bir_lowering=False)

    # ---- DRAM I/O (per-core shard, feature-major) ----
    d_xin = nc.dram_tensor("xin", [XIN, BS], BF16, kind="ExternalInput").ap()
    d_hcon = nc.dram_tensor("hcon", [CON, BS], BF16, kind="ExternalInput").ap()
    d_hgen = nc.dram_tensor("hgen", [GEN, BS], BF16, kind="ExternalInput").ap()
    d_hgen8 = nc.dram_tensor("hgen8", [GEN, BS], FP8, kind="ExternalInput").ap()
    d_eps = nc.dram_tensor("epsT", [CO, BS], BF16, kind="ExternalInput").ap()
    d_wcih = nc.dram_tensor("wcih", [XIN, 3 * CON], BF16, kind="ExternalInput").ap()
    d_wchh = nc.dram_tensor("wchh", [CON, 3 * CON], BF16, kind="ExternalInput").ap()
    d_wgih = nc.dram_tensor("wgih2", [128, 3 * GEN], BF16, kind="ExternalInput").ap()
    d_wghh8 = nc.dram_tensor("wghh8", [GEN, 3 * GEN], FP8, kind="ExternalInput").ap()
    d_wco = nc.dram_tensor("wco2", [CON, 2 * CO], BF16, kind="ExternalInput").ap()
    d_wfac8 = nc.dram_tensor("wfac8", [GEN, FAC], FP8, kind="ExternalInput").ap()
    d_bias = nc.dram_tensor("bias", [128, 20], F32, kind="ExternalInput").ap()
    d_out = nc.dram_tensor("out", [1088, BS], BF16, kind="ExternalOutput").ap()

    with tile.TileContext(nc) as tc, ExitStack() as ctx:
        wpool = ctx.enter_context(tc.tile_pool(name="w", bufs=1))
        iop = ctx.enter_context(tc.tile_pool(name="io", bufs=kn["io_bufs"]))
        mid = ctx.enter_context(tc.tile_pool(name="mid", bufs=1))
        outp = ctx.enter_context(tc.tile_pool(name="out", bufs=2))
        outp1 = ctx.enter_context(tc.tile_pool(name="out1", bufs=2))
        pcon = ctx.enter_context(tc.tile_pool(name="pcon", bufs=kn["pcon"], space="PSUM"))
        pgen = ctx.enter_context(tc.tile_pool(name="pgen", bufs=kn["pgen"], space="PSUM"))
        psco = ctx.enter_context(tc.tile_pool(name="psco", bufs=kn["psco"], space="PSUM"))

        # ---- weights (persistent) ----
        wcih = wpool.tile([128, 3, 3 * CON], BF16, tag="wcih")
        nc.sync.dma_start(wcih[:], d_wcih.rearrange("(k p) m -> p k m", p=128))
        wchh = wpool.tile([128, 2, 3 * CON], BF16, tag="wchh")
        nc.sync.dma_start(wchh[:], d_wchh.rearrange("(k p) m -> p k m", p=128))
        wgih2 = wpool.tile([128, 3 * GEN], BF16, tag="wgih2")
        nc.sync.dma_start(wgih2[:], d_wgih)
        wghh8 = wpool.tile([128, 4, 3 * GEN], FP8, tag="wghh8")
        nc.sync.dma_start(wghh8[:], d_wghh8.rearrange("(k p) m -> p k m", p=128))
        wco2 = wpool.tile([128, 2, 2 * CO], BF16, tag="wco2")
        nc.sync.dma_start(wco2[:], d_wco.rearrange("(k p) m -> p k m", p=128))
        wfac8 = wpool.tile([128, 4, FAC], FP8, tag="wfac8")
        nc.sync.dma_start(wfac8[:], d_wfac8.rearrange("(k p) m -> p k m", p=128))
        tb = wpool.tile([128, 20], F32, tag="bias")
        nc.sync.dma_start(tb[:], d_bias)

        r_xin = d_xin.rearrange("(k p) n -> p k n", p=128)
        r_hcon = d_hcon.rearrange("(k p) n -> p k n", p=128)
        r_hgen = d_hgen.rearrange("(k p) n -> p k n", p=128)
        r_hgen8 = d_hgen8.rearrange("(k p) n -> p k n", p=128)
        r_ogen = d_out[0:GEN, :].rearrange("(k p) n -> p k n", p=128)
        r_ocon = d_out[GEN:GEN + CON, :].rearrange("(k p) n -> p k n", p=128)

        for rep in range(reps):
          for t in range(NTILES):
            cs = slice(t * NT, (t + 1) * NT)

            # ---- loads ----
            txin = iop.tile([128, 3, NT], BF16, tag="xin")
            nc.sync.dma_start(txin[:], r_xin[:, :, cs])
            thcon = iop.tile([128, 2, NT], BF16, tag="hcon")
            nc.sync.dma_start(thcon[:], r_hcon[:, :, cs])
            thgen = iop.tile([128, 4, NT], BF16, tag="hgen")
            nc.sync.dma_start(thgen[:], r_hgen[:, :, cs])
            thgen8 = iop.tile([128, 4, NT], FP8, tag="hgen8")
            nc.sync.dma_start(thgen8[:], r_hgen8[:, :, cs])
            teps = iop.tile([CO, NT], BF16, tag="eps")
            nc.sync.dma_start(teps[:], d_eps[:, cs])

            # ---- controller GRU (bf16) ----
            # Tz/Tr = tanh(0.5*(xW+hW) + 0.5*b)  (== 2*sigmoid - 1)
            zc = mid.tile([128, 2, NT], BF16, tag="zc")
            rc = mid.tile([128, 2, NT], BF16, tag="rc")
            rhc = mid.tile([128, 2, NT], BF16, tag="rhc")
            for mb in (2, 3, 0, 1):  # r-gate first: it feeds the n-chain
                p = pcon.tile([128, NT], F32, tag="ps")
                ms = slice(mb * 128, (mb + 1) * 128)
                for k in range(3):
                    nc.tensor.matmul(p[:], wcih[:, k, ms], txin[:, k, :],
                                     start=(k == 0), stop=False)
                for k in range(2):
                    nc.tensor.matmul(p[:], wchh[:, k, ms], thcon[:, k, :],
                                     start=False, stop=(k == 1))
                dst = zc if mb < 2 else rc
                nc.scalar.activation(dst[:, mb % 2, :], p[:], AF.Tanh,
                                     bias=tb[:, mb:mb + 1], scale=0.5)
                if mb >= 2:  # rh' = (Tr+1)*h per k-block, as soon as Tr lands
                    kb = mb - 2
                    nc.vector.scalar_tensor_tensor(
                        rhc[:, kb, :], rc[:, kb, :], 1.0, thcon[:, kb, :],
                        OP.add, OP.mult)
            # n pre-activation: xn part first (rh' part accumulated later)
            pn_list = []
            for mb in range(2):
                p = pcon.tile([128, NT], F32, tag="ps")
                ms = slice(2 * CON + mb * 128, 2 * CON + (mb + 1) * 128)
                for k in range(3):
                    nc.tensor.matmul(p[:], wcih[:, k, ms], txin[:, k, :],
                                     start=(k == 0), stop=False,
                                     skip_group_check=True)
                pn_list.append(p)
            # off-chain: zm = 1-z ; zh' = (Tz+1)*h = 2*z*h
            zmc = mid.tile([128, 2, NT], BF16, tag="zmc")
            getattr(nc, kn["zm_eng"]).tensor_scalar(zmc[:], zc[:], -0.5, 0.5, OP.mult, OP.add)
            zhc = mid.tile([128, 2, NT], BF16, tag="zhc")
            nc.vector.scalar_tensor_tensor(zhc[:], zc[:], 1.0, thcon[:],
                                           OP.add, OP.mult)
            ncn = mid.tile([128, 2, NT], BF16, tag="ncn")
            for mb in range(2):
                p = pn_list[mb]
                ms = slice(2 * CON + mb * 128, 2 * CON + (mb + 1) * 128)
                for k in range(2):
                    nc.tensor.matmul(p[:], wchh[:, k, ms], rhc[:, k, :],
                                     start=False, stop=(k == 1),
                                     skip_group_check=True)
                nc.scalar.activation(ncn[:, mb, :], p[:], AF.Tanh,
                                     bias=tb[:, 4 + mb:5 + mb], scale=1.0)
            # h' = 0.5*zh' + n*zm ; clip -- split per k-block so the packed
            # co matmul can start on block 0 while block 1 combines
            t2c = mid.tile([128, 2, NT], BF16, tag="rhc")
            hc = mid.tile([128, 2, NT], BF16, tag="zc")
            tcs = outp.tile([128, 2, NT], BF16, tag="cs")
            pco = psco.tile([128, NT], F32, tag="ps")
            for kb in range(2):
                nc.vector.tensor_tensor(t2c[:, kb, :], ncn[:, kb, :],
                                        zmc[:, kb, :], OP.mult)
                nc.vector.scalar_tensor_tensor(hc[:, kb, :], zhc[:, kb, :], 0.5,
                                               t2c[:, kb, :], OP.mult, OP.add)
                getattr(nc, kn["clip_eng"]).tensor_scalar(
                    tcs[:, kb, :], hc[:, kb, :], CLIP, -CLIP, OP.min, OP.max)
                # packed co: psum[0:64]=mean, psum[64:128]=0.5*logvar
                nc.tensor.matmul(pco[:], wco2[:, kb, :], tcs[:, kb, :],
                                 start=(kb == 0), stop=(kb == 1),
                                 skip_group_check=True)
            nc.scalar.dma_start(r_ocon[:, :, cs], tcs[:])
            tmean = outp1.tile([CO, NT], BF16, tag="mean")
            nc.vector.tensor_scalar_add(tmean[:], pco[0:CO, :], tb[0:CO, 18:19])
            tstd = outp1.tile([CO, NT], BF16, tag="std")
            nc.scalar.activation(tstd[:], pco[CO:2 * CO, :], AF.Exp,
                                 bias=tb[0:CO, 19:20], scale=1.0)
            nc.scalar.dma_start(d_out[GEN + CON:GEN + CON + CO, cs], tmean[:])
            nc.scalar.dma_start(d_out[GEN + CON + CO:GEN + CON + 2 * CO, cs], tstd[:])
            tse = mid.tile([CO, NT], BF16, tag="se")
            nc.vector.tensor_tensor(tse[:], tstd[:], teps[:], OP.mult)
            # gi duplicated into both partition halves for row-packed ih MMs
            gi2 = outp.tile([128, NT], BF16, tag="gi")
            nc.vector.tensor_tensor(gi2[0:CO, :], tmean[:], tse[:], OP.add)
            getattr(nc, kn["zm_eng"]).tensor_copy(gi2[CO:128, :], gi2[0:CO, :])
            nc.scalar.dma_start(d_out[GEN + CON + 2 * CO:GEN + CON + 3 * CO, cs],
                                gi2[0:CO, :])

            # ---- generator GRU: hh in fp8 DoubleRow, ih bf16 row-packed ----
            zg = mid.tile([128, 4, NT], BF16, tag="zg")
            rg = mid.tile([128, 4, NT], BF16, tag="rg")
            rhg8 = mid.tile([128, 4, NT], FP8, tag="rhg8")
            for mba, mbb in ((4, 5), (6, 7), (0, 1), (2, 3)):  # r-gate first
                pp = []
                for mb in (mba, mbb):
                    p = pgen.tile([128, NT], F32, tag="ps")
                    ms = slice(mb * 128, (mb + 1) * 128)
                    for c in range(2):
                        nc.tensor.matmul(p[:], wghh8[:, 2 * c:2 * c + 2, ms],
                                         thgen8[:, 2 * c:2 * c + 2, :],
                                         start=(c == 0), stop=False,
                                         perf_mode=DR, skip_group_check=True)
                    pp.append(p)
                # row-packed ih pair: halves run concurrently on the PE
                for i, mb in enumerate((mba, mbb)):
                    ms = slice(mb * 128, (mb + 1) * 128)
                    hp = slice(i * CO, (i + 1) * CO)
                    nc.tensor.matmul(pp[i][:], wgih2[hp, ms], gi2[hp, :],
                                     start=False, stop=True,
                                     skip_group_check=True)
                for i, mb in enumerate((mba, mbb)):
                    dst = zg if mb < 4 else rg
                    nc.scalar.activation(dst[:, mb % 4, :], pp[i][:], AF.Tanh,
                                         bias=tb[:, 6 + mb:7 + mb], scale=0.03125)
                    if mb >= 4:  # rh' = (Tr+1)*h -> fp8, as soon as Tr lands
                        kb = mb - 4
                        nc.vector.scalar_tensor_tensor(
                            rhg8[:, kb, :], rg[:, kb, :], 1.0, thgen[:, kb, :],
                            OP.add, OP.mult)
            zmg = mid.tile([128, 4, NT], BF16, tag="zmg")
            getattr(nc, kn["zm_eng"]).tensor_scalar(zmg[:], zg[:], -0.5, 0.5, OP.mult, OP.add)
            zhg = mid.tile([128, 4, NT], BF16, tag="zhg")
            nc.vector.scalar_tensor_tensor(zhg[:], zg[:], 1.0, thgen[:],
                                           OP.add, OP.mult)
            ngn = mid.tile([128, 4, NT], BF16, tag="ngn")
            for mba, mbb in ((0, 1), (2, 3)):
                pp = []
                for i, mb in enumerate((mba, mbb)):
                    p = pgen.tile([128, NT], F32, tag="ps")
                    ms = slice(2 * GEN + mb * 128, 2 * GEN + (mb + 1) * 128)
                    hp = slice(i * CO, (i + 1) * CO)
                    nc.tensor.matmul(p[:], wgih2[hp, ms], gi2[hp, :],
                                     start=True, stop=False,
                                     skip_group_check=True)
                    pp.append(p)
                for i, mb in enumerate((mba, mbb)):
                    ms = slice(2 * GEN + mb * 128, 2 * GEN + (mb + 1) * 128)
                    for c in range(2):
                        nc.tensor.matmul(pp[i][:], wghh8[:, 2 * c:2 * c + 2, ms],
                                         rhg8[:, 2 * c:2 * c + 2, :],
                                         start=False, stop=(c == 1),
                                         perf_mode=DR, skip_group_check=True)
                    nc.scalar.activation(ngn[:, mb, :], pp[i][:], AF.Tanh,
                                         bias=tb[:, 14 + mb:15 + mb], scale=0.0625)
            t2g = mid.tile([128, 4, NT], BF16, tag="t2g")
            nc.vector.tensor_tensor(t2g[:], ngn[:], zmg[:], OP.mult)
            hg = mid.tile([128, 4, NT], BF16, tag="zg")
            nc.vector.scalar_tensor_tensor(hg[:], zhg[:], 0.5, t2g[:],
                                           OP.mult, OP.add)
            tgs = outp.tile([128, 4, NT], BF16, tag="gs")
            getattr(nc, kn["clip_eng"]).tensor_scalar(tgs[:], hg[:], CLIP, -CLIP, OP.min, OP.max)
            tgs8 = outp.tile([128, 4, NT], FP8, tag="gs8")
            getattr(nc, kn["clip_eng"]).tensor_scalar(tgs8[:], hg[:], CLIP, -CLIP, OP.min, OP.max)
            nc.scalar.dma_start(r_ogen[:, :, cs], tgs[:])

            # ---- factors (fp8 DoubleRow) ----
            pf = pgen.tile([FAC, NT], F32, tag="ps")
            for c in range(2):
                nc.tensor.matmul(pf[:], wfac8[:, 2 * c:2 * c + 2, :],
                                 tgs8[:, 2 * c:2 * c + 2, :],
                                 start=(c == 0), stop=(c == 1), perf_mode=DR)
            tfc = outp1.tile([FAC, NT], BF16, tag="fc")
            nc.vector.tensor_scalar(tfc[:], pf[:], 0.0625, None, OP.mult)
            nc.scalar.dma_start(d_out[GEN + CON + 3 * CO:1088, cs], tfc[:])

    nc.compile()
    nc.m = get_hw_module(nc.m)
    return nc


def _prep_inputs(inputs):
    import ml_dtypes
    BF = ml_dtypes.bfloat16
    E4 = ml_dtypes.float8_e4m3
    f32 = np.float32

    def to8(x):
        return np.clip(np.asarray(x, f32), -240.0, 240.0).astype(E4)

    inp = np.asarray(inputs["input"], dtype=f32)
    h0 = np.asarray(inputs["h_0"], dtype=f32)
    eps = np.asarray(inputs["eps"], dtype=f32)

    wcih = np.ascontiguousarray(np.asarray(inputs["con_Wih"], f32).T)
    wchh = np.ascontiguousarray(np.asarray(inputs["con_Whh"], f32).T)
    # n-gate hidden weights halved: rh' = 2*r*h is fed to these matmuls
    wchh[:, 2 * CON:3 * CON] *= 0.5
    # gen weights: x16 scale (fp8 range), n-chunk additionally halved
    wgihT = np.asarray(inputs["gen_Wih"], f32).T * 16.0
    wgih2 = np.ascontiguousarray(np.concatenate([wgihT, wgihT], axis=0))
    wghhT = np.asarray(inputs["gen_Whh"], f32).T * 16.0
    wghhT[:, 2 * GEN:3 * GEN] *= 0.5
    wghh8 = to8(wghhT)
    # co packed: cols 0:64 mean, 64:128 logvar*0.5
    wcoT = np.asarray(inputs["co_W"], f32).T
    wco2 = np.ascontiguousarray(wcoT)
    wco2[:, CO:2 * CO] *= 0.5
    fw = np.asarray(inputs["fac_W"], f32)
    norm = np.sqrt((fw.astype(np.float64) ** 2).sum(axis=1, keepdims=True))
    nw = (fw / np.maximum(norm, 1e-12)).astype(f32)
    wfac8 = to8(np.ascontiguousarray(nw.T) * 16.0)

    bias = np.zeros((128, 20), dtype=f32)
    cb = (np.asarray(inputs["con_bih"], f32) + np.asarray(inputs["con_bhh"], f32))
    gb = (np.asarray(inputs["gen_bih"], f32) + np.asarray(inputs["gen_bhh"], f32))
    cob = np.asarray(inputs["co_b"], f32)
    for mb in range(2):
        # z/r gates go through tanh(0.5*x + 0.5*b) -> store half-bias
        bias[:, mb] = 0.5 * cb[0 * CON + mb * 128:0 * CON + (mb + 1) * 128]
        bias[:, 2 + mb] = 0.5 * cb[1 * CON + mb * 128:1 * CON + (mb + 1) * 128]
        bias[:, 4 + mb] = cb[2 * CON + mb * 128:2 * CON + (mb + 1) * 128]
    for mb in range(4):
        bias[:, 6 + mb] = 0.5 * gb[0 * GEN + mb * 128:0 * GEN + (mb + 1) * 128]
        bias[:, 10 + mb] = 0.5 * gb[1 * GEN + mb * 128:1 * GEN + (mb + 1) * 128]
        bias[:, 14 + mb] = gb[2 * GEN + mb * 128:2 * GEN + (mb + 1) * 128]
    bias[0:CO, 18] = cob[0:CO]
    bias[0:CO, 19] = 0.5 * cob[CO:2 * CO]

    shared = {"wcih": wcih.astype(BF), "wchh": wchh.astype(BF),
              "wgih2": wgih2.astype(BF), "wghh8": wghh8,
              "wco2": wco2.astype(BF), "wfac8": wfac8, "bias": bias}
    hgen_all = h0[:, 0:GEN]
    hcon_all = h0[:, GEN:GEN + CON]
    xin_all = np.concatenate([inp, h0[:, GEN + CON + 3 * CO:1088]], axis=1)
    in_maps = []
    for c in range(NCORES):
        r = slice(c * BS, (c + 1) * BS)
        m = dict(shared)
        m["xin"] = np.ascontiguousarray(xin_all[r].T.astype(BF))
        m["hcon"] = np.ascontiguousarray(hcon_all[r].T.astype(BF))
        hgT = np.ascontiguousarray(hgen_all[r].T)
        m["hgen"] = hgT.astype(BF)
        m["hgen8"] = to8(hgT)
        m["epsT"] = np.ascontiguousarray(eps[r].T.astype(BF))
        in_maps.append(m)
    return in_maps


def _get_nc(reps=1, nt=None, knobs=None):
    key = f"nc{reps}_{nt}_{knobs}"
    if key not in _CACHE:
        _CACHE[key] = _build(reps, nt, knobs)
    return _CACHE[key]


def _run(in_maps, trace=False, **kw):
    from concourse.bass_utils import run_bass_kernel_spmd
    return run_bass_kernel_spmd(_get_nc(), in_maps,
                                core_ids=list(range(NCORES)), trace=trace, **kw)


def kernel(**inputs):
    in_maps = _prep_inputs(inputs)
    res = _run(in_maps)
    out = np.empty((B, 1088), dtype=np.float32)
    for c in range(NCORES):
        out[c * BS:(c + 1) * BS] = res.results[c]["out"].astype(np.float32).T
    return out
